# revision 5
# baseline (speedup 1.0000x reference)
"""DitLinearTemporalSelfAttention on 8 TRN2 NeuronCores (Bass/Tile).

Sharding: token-parallel. Core c handles batch b=c//2, token half c%2
(2048 tokens, full D=1024). The temporal-softmax/context reduction over
T=4096 spans two cores per batch -> pairwise AllReduce [[0,1],[2,3],...]
of the tiny per-batch [H,dh,dh+1] context+ksum buffer (266 KB).

Precision plan (validated vs fp32 reference, rel<7.2e-3):
  k,q proj:  plain fp8e4 DoubleRow matmuls (weights x64 prescale, /64 in
             the Exp epilogue).  Softmax smooths fp8 noise.
  v,out proj: 3-term fp8 split  hi@Wh + lo@Wh + hi@Wl  (bf16-accurate).
  einsums:   bf16 operands, fp32 psum, block-diag 2-head packing.
  gate_msa:  folded into out_W columns host-side (per-core batch const).
LayerNorm rstd = Newton rsqrt on DVE (no scalar sqrt -> the scalar
engine runs one act table per phase: Exp front, Silu back)."""

import numpy as np

import concourse.bass as bass
import concourse.bacc as bacc
import concourse.mybir as mybir
import concourse.tile as tile
from concourse import masks
from concourse.bass_utils import run_bass_kernel_spmd

B, T, D, H, DH = 4, 4096, 1024, 16, 64
NCORES = 8
TL = T // 2          # tokens per core
NT = TL // 128       # 16 token tiles
NG = NT // 2         # 8 two-tile groups
ND = D // 128        # 8 d-chunks
EPS = 1e-5
WS = 64.0            # fp8 weight prescale
FP32 = mybir.dt.float32
FP32R = mybir.dt.float32r
BF16 = mybir.dt.bfloat16
FP8 = mybir.dt.float8e4
I32 = mybir.dt.int32
DR = mybir.MatmulPerfMode.DoubleRow

_CACHE = {}
USE_COLLECTIVE = True


def _legalize_waits(nc, cap=2, escap=2):
    """Split >cap semaphore waits off any instruction into EventSemaphore
    instructions placed immediately before it on the same engine."""
    n = 0
    for bb in nc.main_func.blocks:
        out = []
        changed = False
        for ins in bb.instructions:
            si = ins.sync_info
            ty = type(ins).__name__
            icap = 1 if ty == "InstDMACopy" else cap
            if (si is not None and si.on_wait is not None
                    and len(si.on_wait) > icap
                    and ty not in ("InstDrain", "InstEventSemaphore")):
                waits = list(si.on_wait)
                keep, extra = waits[:icap], waits[icap:]
                while extra:
                    chunk, extra = extra[:escap], extra[escap:]
                    n += 1
                    es = mybir.InstEventSemaphore(
                        name=f"I-wsplit-{n}", engine=ins.engine,
                        sync_info=mybir.SyncInfo(on_wait=list(chunk),
                                                 on_update=[]))
                    out.append(es)
                ins.sync_info = mybir.SyncInfo(
                    on_wait=keep, on_update=list(si.on_update or []))
                changed = True
            out.append(ins)
        if changed:
            bb.instructions = out
    return n


def build():
    from contextlib import ExitStack

    nc = bacc.Bacc("TRN2", target_bir_lowering=False, debug=False,
                   num_devices=NCORES)

    x_d = nc.dram_tensor("x", [TL, D], FP32, kind="ExternalInput")
    xb_d = nc.dram_tensor("xb", [TL, D], BF16, kind="ExternalInput")
    wk_d = nc.dram_tensor("wk8", [D, D], FP8, kind="ExternalInput")
    wq_d = nc.dram_tensor("wq8", [D, D], FP8, kind="ExternalInput")
    wvh_d = nc.dram_tensor("wvh", [D, D], FP8, kind="ExternalInput")
    wvl_d = nc.dram_tensor("wvl", [D, D], FP8, kind="ExternalInput")
    woh_d = nc.dram_tensor("woh", [D, D], FP8, kind="ExternalInput")
    wol_d = nc.dram_tensor("wol", [D, D], FP8, kind="ExternalInput")
    embw_d = nc.dram_tensor("embw", [D, 2 * D], BF16, kind="ExternalInput")
    embt_d = nc.dram_tensor("embt", [D], FP32, kind="ExternalInput")
    gsn_d = nc.dram_tensor("gsn", [2, D], FP32R, kind="ExternalInput")
    out_d = nc.dram_tensor("out", [TL, D], FP32, kind="ExternalOutput")

    def _emit(tc, es):
        constp = es.enter_context(tc.tile_pool(name="const", bufs=1))
        xio = es.enter_context(tc.tile_pool(name="xio", bufs=5))
        statp = es.enter_context(tc.tile_pool(name="stat", bufs=6))
        dramp = es.enter_context(tc.tile_pool(name="dram", bufs=1, space="DRAM"))
        tp = es.enter_context(tc.tile_pool(name="tp", bufs=2, space="PSUM"))
        pp = es.enter_context(tc.tile_pool(name="pp", bufs=4, space="PSUM"))

        # ---------------- constants ----------------
        ident = constp.tile([128, 128], FP32)
        masks.make_identity(nc, ident[:])
        identb = constp.tile([128, 128], BF16)
        nc.vector.tensor_copy(identb[:], ident[:])
        ones_row32 = constp.tile([1, 512], FP32)
        nc.vector.memset(ones_row32[:], 1.0)
        ones_row = constp.tile([1, 512], FP32R)
        nc.vector.tensor_copy(ones_row[:], ones_row32[:])

        gsn = constp.tile([1, 2 * D], FP32R)
        nc.sync.dma_start(out=gsn[:], in_=gsn_d[:].rearrange("a b -> (a b)").unsqueeze(0))
        sng_row = gsn[:, 0:D]
        snb_row = gsn[:, D:2 * D]

        # ---------------- emb MLP (streamed bf16, one-time) ----------------
        es_rows = ExitStack()
        rowsp = es_rows.enter_context(tc.tile_pool(name="rows", bufs=1))
        embp = es_rows.enter_context(tc.tile_pool(name="embw", bufs=2))

        embt_sb = rowsp.tile([128, ND], FP32)
        nc.sync.dma_start(out=embt_sb[:], in_=embt_d[:].rearrange(
            "(p c) -> p c", c=ND))
        # silu(emb) on DVE: e = exp(-x) on scalar would thrash; emb is tiny so
        # use scalar Silu BEFORE the Exp phase begins (one early table load).
        silu_e = rowsp.tile([128, ND], FP32)
        nc.scalar.activation(silu_e[:], embt_sb[:],
                             mybir.ActivationFunctionType.Silu)
        silu_eb = rowsp.tile([128, 2 * ND], BF16)
        nc.vector.tensor_copy(
            silu_eb[:].rearrange("p (c two) -> p c two", two=2)[:, :, 0:1],
            silu_e[:].unsqueeze(2))
        emb_sel = rowsp.tile([1, 2 * D], FP32R)
        for nch in range(4):
            embw = embp.tile([128, ND * 512], BF16, tag="embw")
            nc.sync.dma_start(
                out=embw[:].rearrange("p (dc c) -> p dc c", c=512),
                in_=embw_d[:, nch * 512:(nch + 1) * 512].rearrange(
                    "(dc p) c -> p dc c", p=128))
            epn = pp.tile([1, 512], FP32, tag="pp")
            for dc in range(ND):
                nc.tensor.matmul(epn[:],
                                 silu_eb[:, 2 * dc:2 * dc + 1],
                                 embw[:, dc * 512:(dc + 1) * 512],
                                 start=(dc == 0), stop=(dc == ND - 1))
            nc.vector.tensor_copy(emb_sel[:, nch * 512:(nch + 1) * 512], epn[:])
        # broadcast emb_sel + sng/snb rows to all partitions
        emb_sel_b = rowsp.tile([128, 2 * D], FP32)
        for nch in range(4):
            bp = tp.tile([128, 512], FP32, tag="tp")
            nc.tensor.matmul(bp[:], ones_row[:, 0:128],
                             emb_sel[:, nch * 512:(nch + 1) * 512])
            nc.vector.tensor_copy(emb_sel_b[:, nch * 512:(nch + 1) * 512], bp[:])

        def bcast(row, name):
            out = rowsp.tile([128, D], FP32, tag=f"bc_{name}")
            for nh in range(2):
                bp = tp.tile([128, 512], FP32, tag="tp")
                nc.tensor.matmul(bp[:], ones_row[:, 0:128],
                                 row[:, nh * 512:(nh + 1) * 512])
                nc.vector.tensor_copy(out[:, nh * 512:(nh + 1) * 512], bp[:])
            return out

        sng_b = bcast(sng_row, "sng")
        snb_b = bcast(snb_row, "snb")
        # scale2 = sng*(1+scale); shift2 = snb*(1+scale) + shift
        t1_b = rowsp.tile([128, D], FP32)
        nc.vector.tensor_scalar(t1_b[:], emb_sel_b[:, 0:D], 1.0, None,
                                mybir.AluOpType.add)
        s2_b = constp.tile([128, D], FP32)
        nc.vector.tensor_tensor(s2_b[:], t1_b[:], sng_b[:],
                                mybir.AluOpType.mult)
        sh2_b = constp.tile([128, D], FP32)
        nc.vector.tensor_tensor(sh2_b[:], t1_b[:], snb_b[:],
                                mybir.AluOpType.mult)
        nc.vector.tensor_tensor(sh2_b[:], sh2_b[:], emb_sel_b[:, D:2 * D],
                                mybir.AluOpType.add)
        es_rows.close()

        # ---------------- persistent sbuf tensors ----------------
        es_xnt = ExitStack()
        xntp = es_xnt.enter_context(tc.tile_pool(name="xnT", bufs=1))
        xh = xntp.tile([128, ND * TL], FP8)   # xn^T hi, dc-major
        xl = xntp.tile([128, ND * TL], FP8)   # xn^T lo

        es_wkv = ExitStack()
        wkvp = es_wkv.enter_context(tc.tile_pool(name="wkv", bufs=1))
        wk8 = wkvp.tile([128, ND * D], FP8)
        wvh = wkvp.tile([128, ND * D], FP8)
        wvl = wkvp.tile([128, ND * D], FP8)
        for dst, src in ((wk8, wk_d), (wvh, wvh_d), (wvl, wvl_d)):
            nc.sync.dma_start(
                out=dst[:].rearrange("p (dc c) -> p dc c", c=D),
                in_=src[:].rearrange("(dc p) c -> p dc c", p=128))

        kvp = es_wkv.enter_context(tc.tile_pool(name="kv", bufs=2))
        ctx_sb = constp.tile([128, 8 * 65], FP32)

        def newton_rsqrt(rstd, var_ap, tmp_pool, n):
            """rstd = (var + EPS)^-1/2 via magic seed + 2 Newton iters (DVE)."""
            ve_ = tmp_pool.tile([128, n], FP32, tag="nw_ve")
            nc.vector.tensor_scalar(ve_[:], var_ap, EPS, None,
                                    mybir.AluOpType.add)
            vh_ = tmp_pool.tile([128, n], FP32, tag="nw_vh")
            nc.vector.tensor_scalar(vh_[:], ve_[:], 0.5, None,
                                    mybir.AluOpType.mult)
            nc.vector.tensor_scalar(rstd.bitcast(I32), ve_[:].bitcast(I32),
                                    1, None, mybir.AluOpType.arith_shift_right)
            nc.vector.tensor_scalar(rstd.bitcast(I32), rstd.bitcast(I32),
                                    -1, 0x5f3759df,
                                    mybir.AluOpType.mult, mybir.AluOpType.add)
            t1 = tmp_pool.tile([128, n], FP32, tag="nw_t1")
            for _ in range(2):
                nc.vector.tensor_tensor(t1[:], rstd, rstd,
                                        mybir.AluOpType.mult)
                nc.vector.tensor_tensor(t1[:], t1[:], vh_[:],
                                        mybir.AluOpType.mult)
                nc.vector.tensor_scalar(t1[:], t1[:], -1.0, 1.5,
                                        mybir.AluOpType.mult,
                                        mybir.AluOpType.add)
                nc.vector.tensor_tensor(rstd, rstd, t1[:],
                                        mybir.AluOpType.mult)

        # ---------------- front: LN + transpose + kv + ctx ----------------
        xts = {}
        aggs = {}
        rstds = {}
        nmrs = {}

        def s0_stats(t):
            xt = xio.tile([128, D], FP32, tag="xin")
            nc.sync.dma_start(out=xt[:], in_=x_d[t * 128:(t + 1) * 128, :])
            xts[t] = xt
            st6 = statp.tile([128, 2, 6], FP32, tag="st6")
            nc.vector.bn_stats(st6[:, 0, :], xt[:, 0:512])
            nc.vector.bn_stats(st6[:, 1, :], xt[:, 512:1024])
            g = t // 2
            if t % 2 == 0:
                aggs[g] = statp.tile([128, 2, 2], FP32, tag="agg", name="agg")
            nc.vector.bn_aggr(aggs[g][:, t % 2, :], st6[:])

        def newton_front(g):
            rstd2 = statp.tile([128, 2], FP32, tag="rstd")
            newton_rsqrt(rstd2[:], aggs[g][:, :, 1], statp, 2)
            nmr2 = statp.tile([128, 2], FP32, tag="nmr")
            nc.vector.scalar_tensor_tensor(nmr2[:], aggs[g][:, :, 0], -1.0,
                                           rstd2[:], mybir.AluOpType.mult,
                                           mybir.AluOpType.mult)
            rstds[g], nmrs[g] = rstd2, nmr2

        def s0_apply(t):
            g, i = t // 2, t % 2
            xt = xts.pop(t)
            xn = xio.tile([128, D], BF16, tag="xnb")
            nc.scalar.activation(xn[:], xt[:],
                                 mybir.ActivationFunctionType.Identity,
                                 bias=nmrs[g][:, i:i + 1],
                                 scale=rstds[g][:, i:i + 1])
            for grp in range(2):  # groups of 4 d-chunks
                tpt = tp.tile([128, 512], BF16, tag="tp")
                for j in range(4):
                    dc = grp * 4 + j
                    nc.tensor.transpose(tpt[:, j * 128:(j + 1) * 128],
                                        xn[:, dc * 128:(dc + 1) * 128],
                                        identb[:])
                dsth = xh[:].rearrange("p (dc tt) -> p dc tt", tt=TL)[
                    :, grp * 4:(grp + 1) * 4, t * 128:(t + 1) * 128]
                dstl = xl[:].rearrange("p (dc tt) -> p dc tt", tt=TL)[
                    :, grp * 4:(grp + 1) * 4, t * 128:(t + 1) * 128]
                src3 = tpt[:].rearrange("p (j c) -> p j c", c=128)
                nc.scalar.copy(dsth, src3)
                nc.vector.tensor_tensor(dstl, src3, dsth,
                                        mybir.AluOpType.subtract)

        def em_kv(t):
            ke = kvp.tile([128, D], BF16, tag="ke")
            va = kvp.tile([128, H * 66], BF16, tag="va")
            xh3 = xh[:].rearrange("p (dc tt) -> p dc tt", tt=TL)
            xl3 = xl[:].rearrange("p (dc tt) -> p dc tt", tt=TL)
            ts_, te_ = t * 128, (t + 1) * 128
            wk3 = wk8[:].rearrange("p (dc c) -> p dc c", c=D)
            wvh3 = wvh[:].rearrange("p (dc c) -> p dc c", c=D)
            wvl3 = wvl[:].rearrange("p (dc c) -> p dc c", c=D)
            for jh in range(2):
                kh = pp.tile([128, 512], FP32, tag="pp")
                for a in range(4):
                    nc.tensor.matmul(
                        kh[:], xh3[:, 2 * a:2 * a + 2, ts_:te_],
                        wk3[:, 2 * a:2 * a + 2, jh * 512:(jh + 1) * 512],
                        start=(a == 0), stop=(a == 3), perf_mode=DR)
                nc.scalar.activation(ke[:, jh * 512:(jh + 1) * 512], kh[:],
                                     mybir.ActivationFunctionType.Exp,
                                     scale=1.0 / WS)
            for jh in range(2):
                vh_ = pp.tile([128, 512], FP32, tag="pp")
                for a in range(4):
                    nc.tensor.matmul(
                        vh_[:], xh3[:, 2 * a:2 * a + 2, ts_:te_],
                        wvh3[:, 2 * a:2 * a + 2, jh * 512:(jh + 1) * 512],
                        start=(a == 0), stop=False, perf_mode=DR)
                for a in range(4):
                    nc.tensor.matmul(
                        vh_[:], xl3[:, 2 * a:2 * a + 2, ts_:te_],
                        wvh3[:, 2 * a:2 * a + 2, jh * 512:(jh + 1) * 512],
                        start=False, stop=False, perf_mode=DR)
                for a in range(4):
                    nc.tensor.matmul(
                        vh_[:], xh3[:, 2 * a:2 * a + 2, ts_:te_],
                        wvl3[:, 2 * a:2 * a + 2, jh * 512:(jh + 1) * 512],
                        start=False, stop=(a == 3), perf_mode=DR)
                nc.scalar.activation(
                    va[:].rearrange("p (h l) -> p h l", l=66)[
                        :, jh * 8:(jh + 1) * 8, 0:64],
                    vh_[:].rearrange("p (h l) -> p h l", l=64),
                    mybir.ActivationFunctionType.Copy, scale=1.0 / WS)
            nc.vector.memset(
                va[:].rearrange("p (h l) -> p h l", l=66)[:, :, 64:65], 1.0)
            for hp in range(H // 2):
                cp = tp.tile([128, 512], FP32, tag="tp")
                for par in range(2):
                    h = 2 * hp + par
                    nc.tensor.matmul(cp[par * 64:par * 64 + 64, 0:65],
                                     ke[:, h * 64:(h + 1) * 64],
                                     va[:, h * 66:h * 66 + 65])
                if t == 0:
                    nc.vector.tensor_copy(
                        ctx_sb[:, hp * 65:(hp + 1) * 65], cp[:, 0:65])
                else:
                    nc.vector.tensor_tensor(
                        ctx_sb[:, hp * 65:(hp + 1) * 65],
                        ctx_sb[:, hp * 65:(hp + 1) * 65],
                        cp[:, 0:65], mybir.AluOpType.add)

        for g in range(NG + 1):
            if g < NG:
                s0_stats(2 * g)
                s0_stats(2 * g + 1)
                newton_front(g)
                s0_apply(2 * g)
                s0_apply(2 * g + 1)
            if g >= 1:
                em_kv(2 * (g - 1))
                em_kv(2 * (g - 1) + 1)

        # ---------------- ctx AllReduce (pairwise per batch) ----------------
        cc_in = dramp.tile([H, 64, 65], FP32)
        cc_out = dramp.tile([H, 64, 65], FP32)
        for q in range(2):
            nc.sync.dma_start(
                out=cc_in[:].rearrange("(g q) d l -> q d g l", q=2)[q],
                in_=ctx_sb[q * 64:(q + 1) * 64, :].rearrange(
                    "d (g l) -> d g l", l=65))
        if USE_COLLECTIVE:
            nc.gpsimd.collective_compute(
                "AllReduce", mybir.AluOpType.add,
                replica_groups=[[0, 1], [2, 3], [4, 5], [6, 7]],
                ins=[cc_in.opt()], outs=[cc_out.opt()])
        else:
            nc.sync.dma_start(out=cc_out[:], in_=cc_in[:])
        es_wkv.close()

        # ---------------- q proj (overlaps collective) ----------------
        es_wq = ExitStack()
        es_qt = ExitStack()
        wqp = es_wq.enter_context(tc.tile_pool(name="wq", bufs=1))
        qtp = es_qt.enter_context(tc.tile_pool(name="qT", bufs=1, side="right"))
        wq8 = wqp.tile([128, ND * D], FP8)
        nc.sync.dma_start(
            out=wq8[:].rearrange("p (dc c) -> p dc c", c=D),
            in_=wq_d[:].rearrange("(dc p) c -> p dc c", p=128))
        qT = qtp.tile([128, ND * TL], BF16)  # j-chunk jc at cols jc*TL
        wq3 = wq8[:].rearrange("p (dc c) -> p dc c", c=D)
        xh3 = xh[:].rearrange("p (dc tt) -> p dc tt", tt=TL)
        for jc in range(ND):
            for th in range(TL // 512):
                qps = pp.tile([128, 512], FP32, tag="pp")
                for a in range(4):
                    nc.tensor.matmul(
                        qps[:],
                        wq3[:, 2 * a:2 * a + 2, jc * 128:(jc + 1) * 128],
                        xh3[:, 2 * a:2 * a + 2, th * 512:(th + 1) * 512],
                        start=(a == 0), stop=(a == 3), perf_mode=DR)
                nc.scalar.activation(
                    qT[:, jc * TL + th * 512:jc * TL + (th + 1) * 512],
                    qps[:], mybir.ActivationFunctionType.Exp, scale=1.0 / WS)
        es_wq.close()
        es_xnt.close()

        # ---------------- normalize ctx into block-diag bf16 ----------------
        ctxn = constp.tile([128, 8 * 65], FP32)
        for q in range(2):
            nc.sync.dma_start(
                out=ctxn[q * 64:(q + 1) * 64, :].rearrange(
                    "d (g l) -> d g l", l=65),
                in_=cc_out[:].rearrange("(g q) d l -> q d g l", q=2)[q])
        rk = statp.tile([128, 8], FP32, tag="rk")
        nc.vector.reciprocal(
            rk[:], ctxn[:].rearrange("p (g l) -> p g l", l=65)[:, :, 64])
        # ctx_blk: per head-pair hp a [128, 130] block-diagonal tile:
        #   rows 0:64   = ctx head 2hp   -> cols 0:64,   ones col 64
        #   rows 64:128 = ctx head 2hp+1 -> cols 65:129, ones col 129
        ctx_blk = constp.tile([128, 8 * 130], BF16)
        nc.vector.memset(ctx_blk[:], 0.0)
        for hp in range(8):
            for par in range(2):
                nc.vector.tensor_scalar(
                    ctx_blk[par * 64:(par + 1) * 64,
                            hp * 130 + par * 65:hp * 130 + par * 65 + 64],
                    ctxn[par * 64:(par + 1) * 64, hp * 65:hp * 65 + 64],
                    rk[par * 64:(par + 1) * 64, hp:hp + 1], None,
                    mybir.AluOpType.mult)
                nc.vector.memset(
                    ctx_blk[par * 64:(par + 1) * 64,
                            hp * 130 + par * 65 + 64:hp * 130 + par * 65 + 65],
                    1.0)

        # ---------------- back: y einsum + LN + silu + out proj ----------------
        es_out = ExitStack()
        wop = es_out.enter_context(tc.tile_pool(name="wo", bufs=1, side="right"))
        hyp = es_out.enter_context(tc.tile_pool(name="hy", bufs=3, side="right"))
        woh = wop.tile([128, ND * D], FP8)
        wol = wop.tile([128, ND * D], FP8)
        for dst, src in ((woh, woh_d), (wol, wol_d)):
            nc.sync.dma_start(
                out=dst[:].rearrange("p (dc c) -> p dc c", c=D),
                in_=src[:].rearrange("(dc p) c -> p dc c", p=128))
        woh3 = woh[:].rearrange("p (dc c) -> p dc c", c=D)
        wol3 = wol[:].rearrange("p (dc c) -> p dc c", c=D)

        yts, aggy, rstdy, nmry, hss = {}, {}, {}, {}, {}

        def em_y(t):
            yt = hyp.tile([128, D], FP32, tag="yt")
            yts[t] = yt
            for hp in range(8):
                yp = pp.tile([128, 512], FP32, tag="pp")
                nc.tensor.matmul(
                    yp[:, 0:130],
                    qT[:, hp * TL + t * 128:hp * TL + (t + 1) * 128],
                    ctx_blk[:, hp * 130:(hp + 1) * 130])
                rq2 = statp.tile([128, 2], FP32, tag="rq")
                nc.vector.reciprocal(
                    rq2[:], yp[:, 0:130].rearrange(
                        "p (two l) -> p two l", l=65)[:, :, 64])
                nc.vector.tensor_scalar(yt[:, (2 * hp) * 64:(2 * hp) * 64 + 64],
                                        yp[:, 0:64], rq2[:, 0:1], None,
                                        mybir.AluOpType.mult)
                nc.scalar.mul(yt[:, (2 * hp + 1) * 64:(2 * hp + 1) * 64 + 64],
                              yp[:, 65:129], rq2[:, 1:2])

        def stats_y(t):
            yt = yts[t]
            st6 = statp.tile([128, 2, 6], FP32, tag="st6")
            nc.vector.bn_stats(st6[:, 0, :], yt[:, 0:512])
            nc.vector.bn_stats(st6[:, 1, :], yt[:, 512:1024])
            g = t // 2
            if t % 2 == 0:
                aggy[g] = statp.tile([128, 2, 2], FP32, tag="aggy", name="aggy")
            nc.vector.bn_aggr(aggy[g][:, t % 2, :], st6[:])

        def newton_back(g):
            rstd2 = statp.tile([128, 2], FP32, tag="rstdy")
            newton_rsqrt(rstd2[:], aggy[g][:, :, 1], statp, 2)
            nmr2 = statp.tile([128, 2], FP32, tag="nmry")
            nc.vector.scalar_tensor_tensor(nmr2[:], aggy[g][:, :, 0], -1.0,
                                           rstd2[:], mybir.AluOpType.mult,
                                           mybir.AluOpType.mult)
            rstdy[g], nmry[g] = rstd2, nmr2

        def em_ln(t):
            g, i = t // 2, t % 2
            yt = yts.pop(t)
            ln = hyp.tile([128, D], FP32, tag="ln")
            nc.scalar.activation(ln[:], yt[:],
                                 mybir.ActivationFunctionType.Identity,
                                 bias=nmry[g][:, i:i + 1],
                                 scale=rstdy[g][:, i:i + 1])
            nc.gpsimd.tensor_tensor(ln[:], ln[:], s2_b[:],
                                    mybir.AluOpType.mult)
            nc.gpsimd.tensor_tensor(ln[:], ln[:], sh2_b[:],
                                    mybir.AluOpType.add)
            hs = hyp.tile([128, D], BF16, tag="hs")
            nc.scalar.activation(hs[:], ln[:],
                                 mybir.ActivationFunctionType.Silu)
            hss[t] = hs

        def em_out(t):
            hs = hss.pop(t)
            hh = hyp.tile([128, D], FP8, tag="hh")
            hl = hyp.tile([128, D], FP8, tag="hl")
            for grp in range(2):
                tpt = tp.tile([128, 512], BF16, tag="tp")
                for j in range(4):
                    dc = grp * 4 + j
                    nc.tensor.transpose(tpt[:, j * 128:(j + 1) * 128],
                                        hs[:, dc * 128:(dc + 1) * 128],
                                        identb[:])
                nc.scalar.copy(hh[:, grp * 512:(grp + 1) * 512], tpt[:])
                nc.vector.tensor_tensor(hl[:, grp * 512:(grp + 1) * 512],
                                        tpt[:], hh[:, grp * 512:(grp + 1) * 512],
                                        mybir.AluOpType.subtract)
            xbt = xio.tile([128, D], BF16, tag="xbt")
            nc.sync.dma_start(out=xbt[:], in_=xb_d[t * 128:(t + 1) * 128, :])
            fin = xio.tile([128, D], FP32, tag="fin")
            hh3 = hh[:].rearrange("p (dc c) -> p dc c", c=128)
            hl3 = hl[:].rearrange("p (dc c) -> p dc c", c=128)
            for jh in range(2):
                oph = pp.tile([128, 512], FP32, tag="pp")
                for a in range(4):
                    nc.tensor.matmul(
                        oph[:], hh3[:, 2 * a:2 * a + 2, :],
                        woh3[:, 2 * a:2 * a + 2, jh * 512:(jh + 1) * 512],
                        start=(a == 0), stop=False, perf_mode=DR)
                for a in range(4):
                    nc.tensor.matmul(
                        oph[:], hl3[:, 2 * a:2 * a + 2, :],
                        woh3[:, 2 * a:2 * a + 2, jh * 512:(jh + 1) * 512],
                        start=False, stop=False, perf_mode=DR)
                for a in range(4):
                    nc.tensor.matmul(
                        oph[:], hh3[:, 2 * a:2 * a + 2, :],
                        wol3[:, 2 * a:2 * a + 2, jh * 512:(jh + 1) * 512],
                        start=False, stop=(a == 3), perf_mode=DR)
                # fin = oph/WS + x  (one DVE op: psum read + residual)
                nc.vector.scalar_tensor_tensor(
                    fin[:, jh * 512:(jh + 1) * 512], oph[:], 1.0 / WS,
                    xbt[:, jh * 512:(jh + 1) * 512],
                    mybir.AluOpType.mult, mybir.AluOpType.add)
            nc.sync.dma_start(out=out_d[t * 128:(t + 1) * 128, :], in_=fin[:])

        for g in range(NG + 1):
            if g < NG:
                em_y(2 * g)
                em_y(2 * g + 1)
                stats_y(2 * g)
                stats_y(2 * g + 1)
                newton_back(g)
            if g >= 1:
                em_ln(2 * (g - 1))
                em_out(2 * (g - 1))
                em_ln(2 * (g - 1) + 1)
                em_out(2 * (g - 1) + 1)
        es_out.close()
        es_qt.close()

    from contextlib import ExitStack
    with tile.TileContext(nc) as tc, ExitStack() as es:
        _emit(tc, es)
    nc.compile()
    _legalize_waits(nc)
    return nc


def kernel(**inputs):
    x = np.asarray(inputs["x"], np.float32)
    emb = np.asarray(inputs["emb"], np.float32)
    gate_msa = np.asarray(inputs["gate_msa"], np.float32)
    norm_g = np.asarray(inputs["norm_g"], np.float32)
    norm_b = np.asarray(inputs["norm_b"], np.float32)
    Wq = np.asarray(inputs["Wq"], np.float32)
    bq = np.asarray(inputs["bq"], np.float32)
    Wk = np.asarray(inputs["Wk"], np.float32)
    bk = np.asarray(inputs["bk"], np.float32)
    Wv = np.asarray(inputs["Wv"], np.float32)
    bv = np.asarray(inputs["bv"], np.float32)
    emb_W = np.asarray(inputs["emb_W"], np.float32)
    emb_b = np.asarray(inputs["emb_b"], np.float32)
    sn_g = np.asarray(inputs["sn_g"], np.float32)
    sn_b = np.asarray(inputs["sn_b"], np.float32)
    out_W = np.asarray(inputs["out_W"], np.float32)
    out_b = np.asarray(inputs["out_b"], np.float32)

    import ml_dtypes
    E4 = ml_dtypes.float8_e4m3

    # biases must be zero for the fast path (norm_b=0 makes cq/ck/cv zero)
    cq = norm_b @ Wq + bq
    ck = norm_b @ Wk + bk
    cv = norm_b @ Wv + bv
    assert not (np.any(cq) or np.any(ck) or np.any(cv) or np.any(out_b)
                or np.any(emb_b)), "fast path requires zero effective biases"

    # fold layernorm affine into projection weights
    wq_f = norm_g[:, None] * Wq
    wk_f = norm_g[:, None] * Wk
    wv_f = norm_g[:, None] * Wv

    def split8(w):
        hi = (WS * w).astype(E4)
        lo = (WS * w - hi.astype(np.float32)).astype(E4)
        return np.ascontiguousarray(hi), np.ascontiguousarray(lo)

    wk8 = np.ascontiguousarray((WS * wk_f).astype(E4))
    wq8 = np.ascontiguousarray((WS * wq_f).astype(E4))
    wvh, wvl = split8(wv_f)
    embw_bf = emb_W.astype(ml_dtypes.bfloat16)

    if "nc" not in _CACHE:
        _CACHE["nc"] = build()
    nc = _CACHE["nc"]

    in_maps = []
    for c in range(NCORES):
        b, half = c // 2, c % 2
        xs = np.ascontiguousarray(x[b, half * TL:(half + 1) * TL, :])
        wo_g = out_W * gate_msa[b, 0][None, :]   # fold gate into out proj
        woh, wol = split8(wo_g)
        m = {
            "x": xs,
            "xb": xs.astype(ml_dtypes.bfloat16),
            "wk8": wk8, "wq8": wq8, "wvh": wvh, "wvl": wvl,
            "woh": woh, "wol": wol,
            "embw": embw_bf,
            "embt": np.ascontiguousarray(emb[b, 0, :].reshape(ND, 128).T.ravel()),
            "gsn": np.ascontiguousarray(np.stack([sn_g, sn_b])),
        }
        in_maps.append(m)

    res = run_bass_kernel_spmd(nc, in_maps, core_ids=list(range(NCORES)),
                               **_RUN_KW)
    kernel.last_result = res
    out = np.stack([res.results[c]["out"] for c in range(NCORES)])
    return out.reshape(B, 2, TL, D).reshape(B, T, D)


_RUN_KW = {}
kernel.last_result = None


# revision 8
# speedup vs baseline: 1.1683x; 1.1683x over previous
"""DitLinearTemporalSelfAttention on 8 TRN2 NeuronCores (Bass/Tile).

Sharding: token-parallel. Core c handles batch b=c//2, token half c%2
(2048 tokens, full D=1024). The temporal-softmax/context reduction over
T=4096 spans two cores per batch -> pairwise AllReduce [[0,1],[2,3],...]
of the tiny per-batch [H,dh,dh+1] context+ksum buffer.

Key structure (validated vs fp32 reference, rel ~7e-3):
  x is passed both [t,d] (bf16: stats + residual) and host-transposed
  [d,t] (fp8 for k/q DoubleRow, bf16 for v).  LayerNorm is applied in
  the projection epilogue:  proj = rstd[t] * (x@W - mu[t]*colsum(W)),
  the rank-1 mu term as one extra 1-deep matmul per psum tile and the
  rstd[t] as the activation scale operand.  This deletes the whole
  on-chip xn-transpose pipeline.
  k,q proj: fp8e4 DoubleRow (2x PE rate), weights x64, /64 in epilogue.
  v,out proj: bf16 (fp8 fails accuracy, 3-term split is slower on hw).
  einsums: bf16, fp32 psum; y uses block-diag 2-head packed rhs.
  gate_msa folded into out_W host-side.  q-softmax denominator via the
  ones column of ctx_blk, reciprocal+scale on DVE/Act.
  LN rstd = Newton rsqrt on DVE; scalar engine act tables: Exp-only in
  the front, Silu-only in the back (zero table thrash)."""

import numpy as np

import concourse.bass as bass
import concourse.bacc as bacc
import concourse.mybir as mybir
import concourse.tile as tile
from concourse import masks
from concourse.bass_utils import run_bass_kernel_spmd

B, T, D, H, DH = 4, 4096, 1024, 16, 64
NCORES = 8
TL = T // 2          # tokens per core
NT = TL // 128       # 16 token tiles
NG = NT // 2         # 8 two-tile groups
ND = D // 128        # 8 d-chunks
EPS = 1e-5
WS = 64.0            # fp8 weight prescale
FP32 = mybir.dt.float32
FP32R = mybir.dt.float32r
BF16 = mybir.dt.bfloat16
FP8 = mybir.dt.float8e4
I32 = mybir.dt.int32
DR = mybir.MatmulPerfMode.DoubleRow

_CACHE = {}
USE_COLLECTIVE = True


def _legalize_waits(nc, cap=2, escap=2):
    """Split >cap semaphore waits off any instruction into EventSemaphore
    instructions placed immediately before it on the same engine."""
    n = 0
    for bb in nc.main_func.blocks:
        out = []
        changed = False
        for ins in bb.instructions:
            si = ins.sync_info
            ty = type(ins).__name__
            icap = 1 if ty == "InstDMACopy" else cap
            if (si is not None and si.on_wait is not None
                    and len(si.on_wait) > icap
                    and ty not in ("InstDrain", "InstEventSemaphore")):
                waits = list(si.on_wait)
                keep, extra = waits[:icap], waits[icap:]
                while extra:
                    chunk, extra = extra[:escap], extra[escap:]
                    n += 1
                    es = mybir.InstEventSemaphore(
                        name=f"I-wsplit-{n}", engine=ins.engine,
                        sync_info=mybir.SyncInfo(on_wait=list(chunk),
                                                 on_update=[]))
                    out.append(es)
                ins.sync_info = mybir.SyncInfo(
                    on_wait=keep, on_update=list(si.on_update or []))
                changed = True
            out.append(ins)
        if changed:
            bb.instructions = out
    return n


def build():
    from contextlib import ExitStack

    nc = bacc.Bacc("TRN2", target_bir_lowering=False, debug=False,
                   num_devices=NCORES)

    xbf_d = nc.dram_tensor("xbf", [TL, D], BF16, kind="ExternalInput")
    xt8_d = nc.dram_tensor("xt8", [D, TL], FP8, kind="ExternalInput")
    xtb_d = nc.dram_tensor("xtb", [D, TL], BF16, kind="ExternalInput")
    wk_d = nc.dram_tensor("wk8", [D, D], FP8, kind="ExternalInput")
    wq_d = nc.dram_tensor("wq8", [D, D], FP8, kind="ExternalInput")
    wv_d = nc.dram_tensor("wvb", [D, D], BF16, kind="ExternalInput")
    wo_d = nc.dram_tensor("wob", [D, D], BF16, kind="ExternalInput")
    csn_d = nc.dram_tensor("csn", [3 * D], BF16, kind="ExternalInput")
    embw_d = nc.dram_tensor("embw", [D, 2 * D], BF16, kind="ExternalInput")
    embt_d = nc.dram_tensor("embt", [D], FP32, kind="ExternalInput")
    gsn_d = nc.dram_tensor("gsn", [2, D], FP32R, kind="ExternalInput")
    out_d = nc.dram_tensor("out", [TL, D], FP32, kind="ExternalOutput")

    def _emit(tc, es):
        constp = es.enter_context(tc.tile_pool(name="const", bufs=1))
        xio = es.enter_context(tc.tile_pool(name="xio", bufs=4))
        statp = es.enter_context(tc.tile_pool(name="stat", bufs=6))
        dramp = es.enter_context(tc.tile_pool(name="dram", bufs=1, space="DRAM"))
        tp = es.enter_context(tc.tile_pool(name="tp", bufs=2, space="PSUM"))
        pp = es.enter_context(tc.tile_pool(name="pp", bufs=4, space="PSUM"))
        cpp = es.enter_context(tc.tile_pool(name="cpp", bufs=2, space="PSUM"))

        # ---------------- constants ----------------
        ident = constp.tile([128, 128], FP32)
        masks.make_identity(nc, ident[:])
        identb = constp.tile([128, 128], BF16)
        nc.vector.tensor_copy(identb[:], ident[:])
        ones_row32 = constp.tile([1, 512], FP32)
        nc.vector.memset(ones_row32[:], 1.0)
        ones_row = constp.tile([1, 512], FP32R)
        nc.vector.tensor_copy(ones_row[:], ones_row32[:])

        gsn = constp.tile([1, 2 * D], FP32R)
        nc.sync.dma_start(out=gsn[:], in_=gsn_d[:].rearrange("a b -> (a b)").unsqueeze(0))
        sng_row = gsn[:, 0:D]
        snb_row = gsn[:, D:2 * D]
        csn = constp.tile([1, 3 * D], BF16)
        nc.sync.dma_start(out=csn[:], in_=csn_d[:].unsqueeze(0))

        rs_all = constp.tile([128, NT], FP32)    # rstd per tile
        rsw_all = constp.tile([128, NT], FP32)   # rstd/WS per tile
        mu_row = constp.tile([1, TL], BF16)      # per-token mean, as a row

        # ---------------- emb MLP (streamed bf16, one-time) ----------------
        es_rows = ExitStack()
        rowsp = es_rows.enter_context(tc.tile_pool(name="rows", bufs=1))
        embp = es_rows.enter_context(tc.tile_pool(name="embw", bufs=2))

        embt_sb = rowsp.tile([128, ND], FP32)
        nc.sync.dma_start(out=embt_sb[:], in_=embt_d[:].rearrange(
            "(p c) -> p c", c=ND))
        silu_e = rowsp.tile([128, ND], FP32)
        nc.scalar.activation(silu_e[:], embt_sb[:],
                             mybir.ActivationFunctionType.Silu)
        silu_eb = rowsp.tile([128, 2 * ND], BF16)
        nc.vector.tensor_copy(
            silu_eb[:].rearrange("p (c two) -> p c two", two=2)[:, :, 0:1],
            silu_e[:].unsqueeze(2))
        emb_sel = rowsp.tile([1, 2 * D], FP32R)
        for nch in range(4):
            embw = embp.tile([128, ND * 512], BF16, tag="embw")
            nc.sync.dma_start(
                out=embw[:].rearrange("p (dc c) -> p dc c", c=512),
                in_=embw_d[:, nch * 512:(nch + 1) * 512].rearrange(
                    "(dc p) c -> p dc c", p=128))
            epn = pp.tile([1, 512], FP32, tag="pp")
            for dc in range(ND):
                nc.tensor.matmul(epn[:],
                                 silu_eb[:, 2 * dc:2 * dc + 1],
                                 embw[:, dc * 512:(dc + 1) * 512],
                                 start=(dc == 0), stop=(dc == ND - 1))
            nc.vector.tensor_copy(emb_sel[:, nch * 512:(nch + 1) * 512], epn[:])
        emb_sel_b = rowsp.tile([128, 2 * D], FP32)
        for nch in range(4):
            bp = tp.tile([128, 512], FP32, tag="tp")
            nc.tensor.matmul(bp[:], ones_row[:, 0:128],
                             emb_sel[:, nch * 512:(nch + 1) * 512])
            nc.vector.tensor_copy(emb_sel_b[:, nch * 512:(nch + 1) * 512], bp[:])

        def bcast(row, name):
            out = rowsp.tile([128, D], FP32, tag=f"bc_{name}")
            for nh in range(2):
                bp = tp.tile([128, 512], FP32, tag="tp")
                nc.tensor.matmul(bp[:], ones_row[:, 0:128],
                                 row[:, nh * 512:(nh + 1) * 512])
                nc.vector.tensor_copy(out[:, nh * 512:(nh + 1) * 512], bp[:])
            return out

        sng_b = bcast(sng_row, "sng")
        snb_b = bcast(snb_row, "snb")
        t1_b = rowsp.tile([128, D], FP32)
        nc.vector.tensor_scalar(t1_b[:], emb_sel_b[:, 0:D], 1.0, None,
                                mybir.AluOpType.add)
        s2_b = constp.tile([128, D], FP32)
        nc.vector.tensor_tensor(s2_b[:], t1_b[:], sng_b[:],
                                mybir.AluOpType.mult)
        sh2_b = constp.tile([128, D], FP32)
        nc.vector.tensor_tensor(sh2_b[:], t1_b[:], snb_b[:],
                                mybir.AluOpType.mult)
        nc.vector.tensor_tensor(sh2_b[:], sh2_b[:], emb_sel_b[:, D:2 * D],
                                mybir.AluOpType.add)
        es_rows.close()

        # ---------------- persistent x^T + front weights ----------------
        es_xt8 = ExitStack()
        xt8p = es_xt8.enter_context(tc.tile_pool(name="xt8", bufs=1))
        xt8 = xt8p.tile([128, ND * TL], FP8)
        nc.sync.dma_start(
            out=xt8[:].rearrange("p (dc t) -> p dc t", t=TL),
            in_=xt8_d[:].rearrange("(dc p) t -> p dc t", p=128))

        es_xtb = ExitStack()
        xtbp = es_xtb.enter_context(tc.tile_pool(name="xtb", bufs=1))
        xtb = xtbp.tile([128, ND * TL], BF16)
        nc.sync.dma_start(
            out=xtb[:].rearrange("p (dc t) -> p dc t", t=TL),
            in_=xtb_d[:].rearrange("(dc p) t -> p dc t", p=128))
        wvb = xtbp.tile([128, ND * D], BF16)
        nc.sync.dma_start(
            out=wvb[:].rearrange("p (dc c) -> p dc c", c=D),
            in_=wv_d[:].rearrange("(dc p) c -> p dc c", p=128))
        wk8 = xtbp.tile([128, ND * D], FP8)
        nc.sync.dma_start(
            out=wk8[:].rearrange("p (dc c) -> p dc c", c=D),
            in_=wk_d[:].rearrange("(dc p) c -> p dc c", p=128))

        kvp = es_xtb.enter_context(tc.tile_pool(name="kv", bufs=2))
        ctx_sb = constp.tile([128, 8 * 65], FP32)

        def newton_rsqrt(rstd, var_ap, tmp_pool, n):
            """rstd = (var + EPS)^-1/2 via magic seed + 2 Newton iters (DVE)."""
            ve_ = tmp_pool.tile([128, n], FP32, tag="nw_ve")
            nc.vector.tensor_scalar(ve_[:], var_ap, EPS, None,
                                    mybir.AluOpType.add)
            vh_ = tmp_pool.tile([128, n], FP32, tag="nw_vh")
            nc.vector.tensor_scalar(vh_[:], ve_[:], 0.5, None,
                                    mybir.AluOpType.mult)
            nc.vector.tensor_scalar(rstd.bitcast(I32), ve_[:].bitcast(I32),
                                    1, None, mybir.AluOpType.arith_shift_right)
            nc.vector.tensor_scalar(rstd.bitcast(I32), rstd.bitcast(I32),
                                    -1, 0x5f3759df,
                                    mybir.AluOpType.mult, mybir.AluOpType.add)
            t1 = tmp_pool.tile([128, n], FP32, tag="nw_t1")
            for _ in range(2):
                nc.vector.tensor_tensor(t1[:], rstd, rstd,
                                        mybir.AluOpType.mult)
                nc.vector.tensor_tensor(t1[:], t1[:], vh_[:],
                                        mybir.AluOpType.mult)
                nc.vector.tensor_scalar(t1[:], t1[:], -1.0, 1.5,
                                        mybir.AluOpType.mult,
                                        mybir.AluOpType.add)
                nc.vector.tensor_tensor(rstd, rstd, t1[:],
                                        mybir.AluOpType.mult)

        # ---------------- front: stats + k/v proj + ctx ----------------
        aggs = {}
        xt83 = xt8[:].rearrange("p (dc t) -> p dc t", t=TL)
        xtb3 = xtb[:].rearrange("p (dc t) -> p dc t", t=TL)
        wk3 = wk8[:].rearrange("p (dc c) -> p dc c", c=D)
        wv3 = wvb[:].rearrange("p (dc c) -> p dc c", c=D)

        def s0_stats(t):
            xt = xio.tile([128, D], BF16, tag="xin")
            nc.sync.dma_start(out=xt[:], in_=xbf_d[t * 128:(t + 1) * 128, :])
            st6 = statp.tile([128, 2, 6], FP32, tag="st6")
            nc.vector.bn_stats(st6[:, 0, :], xt[:, 0:512])
            nc.vector.bn_stats(st6[:, 1, :], xt[:, 512:1024])
            g = t // 2
            if t % 2 == 0:
                aggs[g] = statp.tile([128, 2, 2], FP32, tag="agg", name="agg")
            nc.vector.bn_aggr(aggs[g][:, t % 2, :], st6[:])

        def newton_front(g):
            newton_rsqrt(rs_all[:, 2 * g:2 * g + 2], aggs[g][:, :, 1], statp, 2)
            nc.vector.tensor_scalar(rsw_all[:, 2 * g:2 * g + 2],
                                    rs_all[:, 2 * g:2 * g + 2], 1.0 / WS, None,
                                    mybir.AluOpType.mult)

        def mu_tr(t):
            g, i = t // 2, t % 2
            mp = tp.tile([128, 512], FP32, tag="tp")
            nc.tensor.matmul(mp[0:1, 0:128], aggs[g][:, i, 0:1], ident[:])
            nc.vector.tensor_copy(mu_row[0:1, t * 128:(t + 1) * 128],
                                  mp[0:1, 0:128])

        def em_kv(t):
            ke = kvp.tile([128, D], BF16, tag="ke")
            va = kvp.tile([128, H * 66], BF16, tag="va")
            ts_, te_ = t * 128, (t + 1) * 128
            for jh in range(2):
                kh = pp.tile([128, 512], FP32, tag="pp")
                for a in range(4):
                    nc.tensor.matmul(
                        kh[:], xt83[:, 2 * a:2 * a + 2, ts_:te_],
                        wk3[:, 2 * a:2 * a + 2, jh * 512:(jh + 1) * 512],
                        start=(a == 0), stop=False, perf_mode=DR)
                nc.tensor.matmul(kh[:], mu_row[0:1, ts_:te_],
                                 csn[0:1, jh * 512:(jh + 1) * 512],
                                 start=False, stop=True)
                nc.scalar.activation(ke[:, jh * 512:(jh + 1) * 512], kh[:],
                                     mybir.ActivationFunctionType.Exp,
                                     scale=rsw_all[:, t:t + 1])
            for jh in range(2):
                vh_ = pp.tile([128, 512], FP32, tag="pp")
                for dc in range(ND):
                    nc.tensor.matmul(
                        vh_[:], xtb3[:, dc, ts_:te_],
                        wv3[:, dc, jh * 512:(jh + 1) * 512],
                        start=(dc == 0), stop=False)
                nc.tensor.matmul(vh_[:], mu_row[0:1, ts_:te_],
                                 csn[0:1, 2 * D + jh * 512:2 * D + (jh + 1) * 512],
                                 start=False, stop=True)
                nc.scalar.activation(
                    va[:].rearrange("p (h l) -> p h l", l=66)[
                        :, jh * 8:(jh + 1) * 8, 0:64],
                    vh_[:].rearrange("p (h l) -> p h l", l=64),
                    mybir.ActivationFunctionType.Copy,
                    scale=rs_all[:, t:t + 1])
            nc.vector.memset(
                va[:].rearrange("p (h l) -> p h l", l=66)[:, :, 64:65], 1.0)
            for hp in range(H // 2):
                cp = cpp.tile([128, 512], FP32, tag="cpp")
                for par in range(2):
                    h = 2 * hp + par
                    nc.tensor.matmul(cp[par * 64:par * 64 + 64, 0:65],
                                     ke[:, h * 64:(h + 1) * 64],
                                     va[:, h * 66:h * 66 + 65])
                if t == 0:
                    nc.vector.tensor_copy(
                        ctx_sb[:, hp * 65:(hp + 1) * 65], cp[:, 0:65])
                else:
                    nc.vector.tensor_tensor(
                        ctx_sb[:, hp * 65:(hp + 1) * 65],
                        ctx_sb[:, hp * 65:(hp + 1) * 65],
                        cp[:, 0:65], mybir.AluOpType.add)

        for g in range(NG + 1):
            if g < NG:
                s0_stats(2 * g)
                s0_stats(2 * g + 1)
                newton_front(g)
                mu_tr(2 * g)
                mu_tr(2 * g + 1)
            if g >= 1:
                em_kv(2 * (g - 1))
                em_kv(2 * (g - 1) + 1)

        # ---------------- ctx AllReduce (pairwise per batch) ----------------
        cc_in = dramp.tile([H, 64, 65], FP32)
        cc_out = dramp.tile([H, 64, 65], FP32)
        for q in range(2):
            nc.sync.dma_start(
                out=cc_in[:].rearrange("(g q) d l -> q d g l", q=2)[q],
                in_=ctx_sb[q * 64:(q + 1) * 64, :].rearrange(
                    "d (g l) -> d g l", l=65))
        if USE_COLLECTIVE:
            nc.gpsimd.collective_compute(
                "AllReduce", mybir.AluOpType.add,
                replica_groups=[[0, 1], [2, 3], [4, 5], [6, 7]],
                ins=[cc_in.opt()], outs=[cc_out.opt()])
        else:
            nc.sync.dma_start(out=cc_out[:], in_=cc_in[:])
        es_xtb.close()

        # ---------------- q proj (overlaps collective) ----------------
        es_wq = ExitStack()
        es_qt = ExitStack()
        wqp = es_wq.enter_context(tc.tile_pool(name="wq", bufs=1))
        qtp = es_qt.enter_context(tc.tile_pool(name="qT", bufs=1, side="right"))
        wq8 = wqp.tile([128, ND * D], FP8)
        nc.sync.dma_start(
            out=wq8[:].rearrange("p (dc c) -> p dc c", c=D),
            in_=wq_d[:].rearrange("(dc p) c -> p dc c", p=128))
        qep = es_wq.enter_context(tc.tile_pool(name="qe", bufs=2))
        qT = qtp.tile([128, ND * TL], BF16)  # jc-major [d, t]
        wq3 = wq8[:].rearrange("p (dc c) -> p dc c", c=D)

        def em_q(t):
            ts_, te_ = t * 128, (t + 1) * 128
            qe = qep.tile([128, D], BF16, tag="qe")
            for jh in range(2):
                qp = pp.tile([128, 512], FP32, tag="pp")
                for a in range(4):
                    nc.tensor.matmul(
                        qp[:], xt83[:, 2 * a:2 * a + 2, ts_:te_],
                        wq3[:, 2 * a:2 * a + 2, jh * 512:(jh + 1) * 512],
                        start=(a == 0), stop=False, perf_mode=DR)
                nc.tensor.matmul(qp[:], mu_row[0:1, ts_:te_],
                                 csn[0:1, D + jh * 512:D + (jh + 1) * 512],
                                 start=False, stop=True)
                nc.scalar.activation(qe[:, jh * 512:(jh + 1) * 512], qp[:],
                                     mybir.ActivationFunctionType.Exp,
                                     scale=rsw_all[:, t:t + 1])
            for grp in range(2):
                tpt = tp.tile([128, 512], BF16, tag="tp")
                for j in range(4):
                    dc = grp * 4 + j
                    nc.tensor.transpose(tpt[:, j * 128:(j + 1) * 128],
                                        qe[:, dc * 128:(dc + 1) * 128],
                                        identb[:])
                dst = qT[:].rearrange("p (dc tt) -> p dc tt", tt=TL)[
                    :, grp * 4:(grp + 1) * 4, ts_:te_]
                nc.scalar.copy(dst, tpt[:].rearrange("p (j c) -> p j c", c=128))

        for t in range(NT):
            em_q(t)
        es_wq.close()
        es_xt8.close()

        # ---------------- normalize ctx into block-diag bf16 ----------------
        ctxn = constp.tile([128, 8 * 65], FP32)
        for q in range(2):
            nc.sync.dma_start(
                out=ctxn[q * 64:(q + 1) * 64, :].rearrange(
                    "d (g l) -> d g l", l=65),
                in_=cc_out[:].rearrange("(g q) d l -> q d g l", q=2)[q])
        rk = statp.tile([128, 8], FP32, tag="rk")
        nc.vector.reciprocal(
            rk[:], ctxn[:].rearrange("p (g l) -> p g l", l=65)[:, :, 64])
        ctx_blk = constp.tile([128, 8 * 130], BF16)
        nc.vector.memset(ctx_blk[:], 0.0)
        for hp in range(8):
            for par in range(2):
                nc.vector.tensor_scalar(
                    ctx_blk[par * 64:(par + 1) * 64,
                            hp * 130 + par * 65:hp * 130 + par * 65 + 64],
                    ctxn[par * 64:(par + 1) * 64, hp * 65:hp * 65 + 64],
                    rk[par * 64:(par + 1) * 64, hp:hp + 1], None,
                    mybir.AluOpType.mult)
                nc.vector.memset(
                    ctx_blk[par * 64:(par + 1) * 64,
                            hp * 130 + par * 65 + 64:hp * 130 + par * 65 + 65],
                    1.0)

        # ---------------- back: y einsum + LN + silu + out proj ----------------
        es_out = ExitStack()
        wop = es_out.enter_context(tc.tile_pool(name="wo", bufs=1, side="right"))
        hyp = es_out.enter_context(tc.tile_pool(name="hy", bufs=3, side="right"))
        wob = wop.tile([128, ND * D], BF16)
        nc.sync.dma_start(
            out=wob[:].rearrange("p (dc c) -> p dc c", c=D),
            in_=wo_d[:].rearrange("(dc p) c -> p dc c", p=128))
        wo3 = wob[:].rearrange("p (dc c) -> p dc c", c=D)

        yts, aggy, rstdy, nmry, hss = {}, {}, {}, {}, {}

        def em_y(t):
            yt = hyp.tile([128, D], FP32, tag="yt")
            yts[t] = yt
            for hp in range(8):
                yp = pp.tile([128, 512], FP32, tag="pp")
                nc.tensor.matmul(
                    yp[:, 0:130],
                    qT[:, hp * TL + t * 128:hp * TL + (t + 1) * 128],
                    ctx_blk[:, hp * 130:(hp + 1) * 130])
                rq2 = statp.tile([128, 2], FP32, tag="rq")
                nc.vector.reciprocal(
                    rq2[:], yp[:, 0:130].rearrange(
                        "p (two l) -> p two l", l=65)[:, :, 64])
                nc.vector.tensor_scalar(yt[:, (2 * hp) * 64:(2 * hp) * 64 + 64],
                                        yp[:, 0:64], rq2[:, 0:1], None,
                                        mybir.AluOpType.mult)
                nc.scalar.mul(yt[:, (2 * hp + 1) * 64:(2 * hp + 1) * 64 + 64],
                              yp[:, 65:129], rq2[:, 1:2])

        def stats_y(t):
            yt = yts[t]
            st6 = statp.tile([128, 2, 6], FP32, tag="st6")
            nc.vector.bn_stats(st6[:, 0, :], yt[:, 0:512])
            nc.vector.bn_stats(st6[:, 1, :], yt[:, 512:1024])
            g = t // 2
            if t % 2 == 0:
                aggy[g] = statp.tile([128, 2, 2], FP32, tag="aggy", name="aggy")
            nc.vector.bn_aggr(aggy[g][:, t % 2, :], st6[:])

        def newton_back(g):
            rstd2 = statp.tile([128, 2], FP32, tag="rstdy")
            newton_rsqrt(rstd2[:], aggy[g][:, :, 1], statp, 2)
            nmr2 = statp.tile([128, 2], FP32, tag="nmry")
            nc.vector.scalar_tensor_tensor(nmr2[:], aggy[g][:, :, 0], -1.0,
                                           rstd2[:], mybir.AluOpType.mult,
                                           mybir.AluOpType.mult)
            rstdy[g], nmry[g] = rstd2, nmr2

        def em_ln(t):
            g, i = t // 2, t % 2
            yt = yts.pop(t)
            ln = hyp.tile([128, D], FP32, tag="ln")
            nc.scalar.activation(ln[:], yt[:],
                                 mybir.ActivationFunctionType.Identity,
                                 bias=nmry[g][:, i:i + 1],
                                 scale=rstdy[g][:, i:i + 1])
            nc.gpsimd.tensor_tensor(ln[:], ln[:], s2_b[:],
                                    mybir.AluOpType.mult)
            nc.gpsimd.tensor_tensor(ln[:], ln[:], sh2_b[:],
                                    mybir.AluOpType.add)
            hs = hyp.tile([128, D], BF16, tag="hs")
            nc.scalar.activation(hs[:], ln[:],
                                 mybir.ActivationFunctionType.Silu)
            hss[t] = hs

        def em_out(t):
            hs = hss.pop(t)
            hst = hyp.tile([128, D], BF16, tag="hst")
            for grp in range(2):
                tpt = tp.tile([128, 512], BF16, tag="tp")
                for j in range(4):
                    dc = grp * 4 + j
                    nc.tensor.transpose(tpt[:, j * 128:(j + 1) * 128],
                                        hs[:, dc * 128:(dc + 1) * 128],
                                        identb[:])
                nc.scalar.copy(hst[:, grp * 512:(grp + 1) * 512], tpt[:])
            xbt = xio.tile([128, D], BF16, tag="xbt")
            nc.sync.dma_start(out=xbt[:], in_=xbf_d[t * 128:(t + 1) * 128, :])
            fin = xio.tile([128, D], FP32, tag="fin")
            for jh in range(2):
                oph = pp.tile([128, 512], FP32, tag="pp")
                for dc in range(ND):
                    nc.tensor.matmul(
                        oph[:], hst[:, dc * 128:(dc + 1) * 128],
                        wo3[:, dc, jh * 512:(jh + 1) * 512],
                        start=(dc == 0), stop=(dc == ND - 1))
                nc.vector.tensor_tensor(
                    fin[:, jh * 512:(jh + 1) * 512], oph[:],
                    xbt[:, jh * 512:(jh + 1) * 512], mybir.AluOpType.add)
            nc.sync.dma_start(out=out_d[t * 128:(t + 1) * 128, :], in_=fin[:])

        for g in range(NG + 1):
            if g < NG:
                em_y(2 * g)
                em_y(2 * g + 1)
                stats_y(2 * g)
                stats_y(2 * g + 1)
                newton_back(g)
            if g >= 1:
                em_ln(2 * (g - 1))
                em_out(2 * (g - 1))
                em_ln(2 * (g - 1) + 1)
                em_out(2 * (g - 1) + 1)
        es_out.close()
        es_qt.close()

    from contextlib import ExitStack
    with tile.TileContext(nc) as tc, ExitStack() as es:
        _emit(tc, es)
    nc.compile()
    _legalize_waits(nc)
    return nc


def kernel(**inputs):
    x = np.asarray(inputs["x"], np.float32)
    emb = np.asarray(inputs["emb"], np.float32)
    gate_msa = np.asarray(inputs["gate_msa"], np.float32)
    norm_g = np.asarray(inputs["norm_g"], np.float32)
    norm_b = np.asarray(inputs["norm_b"], np.float32)
    Wq = np.asarray(inputs["Wq"], np.float32)
    bq = np.asarray(inputs["bq"], np.float32)
    Wk = np.asarray(inputs["Wk"], np.float32)
    bk = np.asarray(inputs["bk"], np.float32)
    Wv = np.asarray(inputs["Wv"], np.float32)
    bv = np.asarray(inputs["bv"], np.float32)
    emb_W = np.asarray(inputs["emb_W"], np.float32)
    emb_b = np.asarray(inputs["emb_b"], np.float32)
    sn_g = np.asarray(inputs["sn_g"], np.float32)
    sn_b = np.asarray(inputs["sn_b"], np.float32)
    out_W = np.asarray(inputs["out_W"], np.float32)
    out_b = np.asarray(inputs["out_b"], np.float32)

    import ml_dtypes
    E4 = ml_dtypes.float8_e4m3
    BF = ml_dtypes.bfloat16

    cq = norm_b @ Wq + bq
    ck = norm_b @ Wk + bk
    cv = norm_b @ Wv + bv
    assert not (np.any(cq) or np.any(ck) or np.any(cv) or np.any(out_b)
                or np.any(emb_b)), "fast path requires zero effective biases"

    wq_f = norm_g[:, None] * Wq
    wk_f = norm_g[:, None] * Wk
    wv_f = norm_g[:, None] * Wv

    wk8 = np.ascontiguousarray((WS * wk_f).astype(E4))
    wq8 = np.ascontiguousarray((WS * wq_f).astype(E4))
    wvb = np.ascontiguousarray(wv_f.astype(BF))
    embw_bf = emb_W.astype(BF)
    # negated column sums of the (quantized) weights for the rank-1 mu term
    csn = np.ascontiguousarray(np.stack([
        -wk8.astype(np.float32).sum(0),
        -wq8.astype(np.float32).sum(0),
        -wvb.astype(np.float32).sum(0)]).astype(BF))

    if "nc" not in _CACHE:
        _CACHE["nc"] = build()
    nc = _CACHE["nc"]

    in_maps = []
    for c in range(NCORES):
        b, half = c // 2, c % 2
        xs = np.ascontiguousarray(x[b, half * TL:(half + 1) * TL, :])
        xT = np.ascontiguousarray(xs.T)
        wo_g = out_W * gate_msa[b, 0][None, :]
        m = {
            "xbf": xs.astype(BF),
            "xt8": xT.astype(E4),
            "xtb": xT.astype(BF),
            "wk8": wk8, "wq8": wq8, "wvb": wvb,
            "wob": np.ascontiguousarray(wo_g.astype(BF)),
            "csn": csn,
            "embw": embw_bf,
            "embt": np.ascontiguousarray(emb[b, 0, :].reshape(ND, 128).T.ravel()),
            "gsn": np.ascontiguousarray(np.stack([sn_g, sn_b])),
        }
        in_maps.append(m)

    res = run_bass_kernel_spmd(nc, in_maps, core_ids=list(range(NCORES)),
                               **_RUN_KW)
    kernel.last_result = res
    out = np.stack([res.results[c]["out"] for c in range(NCORES)])
    return out.reshape(B, 2, TL, D).reshape(B, T, D)


_RUN_KW = {}
kernel.last_result = None
